# revision 16
# baseline (speedup 1.0000x reference)
"""MoE routing kernel for Trainium2 (8 NeuronCores, data-parallel over tokens).

Per core (1024 tokens):
  1. Router: scores = x @ Wr.T + br  (fp32 PE matmuls; bias via K=1 ones-row matmul)
  2. Top-2 + softmax (DVE max/max_index, ACT sigmoid)
  3. Per-expert rank of each token via triangular-ones matmuls (prefix counts)
  4. Scatter (token_id, prob) into a capacity-256 slot table (indirect DMA)
  5. Gather x rows per slot tile (fp16), transpose via PE into [din, slots]
  6. Expert matmuls (fp16, fp32 PSUM), scale by prob at PSUM->SBUF copy
  7. Write y to DRAM scratch; per token-tile gather its two expert rows,
     add bias (gates @ be via PE), write out.
Host only shards/concats/transposes.
"""

import os
import numpy as np
from contextlib import ExitStack

PHASES = int(os.environ.get("MOE_PHASES", "4"))  # 1=router, 2=+gather/txp, 3=+expert MM, 4=all
SUB = float(os.environ.get("MOE_SUB", "4"))  # within phase A: 1=scores, 2=+top2/probs, 3=+rank, 4=+scatter

import concourse.bass as bass
import concourse.bacc as bacc
import concourse.mybir as mybir
import concourse.tile as tile
from concourse.masks import make_identity, make_upper_triangular

P = 128
T, DIN, DOUT, E = 8192, 1024, 1024, 16
NCORES = 8
TLOC = T // NCORES          # 1024 tokens per core
NT = TLOC // P              # 8 token tiles
KD = DIN // P               # 8 contraction tiles
CAP = 256                   # per-expert slot capacity (max observed count is 160)
CEFF = E * CAP              # 4096 slots
NS = CEFF // P              # 32 slot tiles
NFREE = 512                 # matmul moving free dim (one PSUM bank of fp32)
ND = DOUT // NFREE          # 2 output chunks

F32 = mybir.dt.float32
I32 = mybir.dt.int32
U32 = mybir.dt.uint32
DT = mybir.dt.float16       # low-precision dtype for expert matmuls
NP_DT = np.float16

AF = mybir.ActivationFunctionType
ALU = mybir.AluOpType


def emit_moe(ctx: ExitStack, tc: "tile.TileContext",
             out_ap, probs_ap, xT, xh, WrT, br, WeT, be):
    nc = tc.nc

    table = nc.dram_tensor("slot_table", [CEFF, 2], I32, kind="Internal").ap()
    y_dram = nc.dram_tensor("y_scratch", [CEFF, DOUT], DT, kind="Internal").ap()

    const = ctx.enter_context(tc.tile_pool(name="const", bufs=1))
    sbig = ctx.enter_context(tc.tile_pool(name="sbig", bufs=1))
    work = ctx.enter_context(tc.tile_pool(name="work", bufs=3))
    xgp = ctx.enter_context(tc.tile_pool(name="xgp", bufs=3))
    wetp = ctx.enter_context(tc.tile_pool(name="wetp", bufs=2))
    yp = ctx.enter_context(tc.tile_pool(name="yp", bufs=3))
    ygp = ctx.enter_context(tc.tile_pool(name="ygp", bufs=2))
    outp = ctx.enter_context(tc.tile_pool(name="outp", bufs=2))
    ps_small = ctx.enter_context(tc.tile_pool(name="ps_small", bufs=2, space="PSUM"))
    ps_txp = ctx.enter_context(tc.tile_pool(name="ps_txp", bufs=2, space="PSUM"))
    ps_y = ctx.enter_context(tc.tile_pool(name="ps_y", bufs=2, space="PSUM"))
    ps_bias = ctx.enter_context(tc.tile_pool(name="ps_bias", bufs=2, space="PSUM"))

    # ---- constants ----
    ones_row = const.tile([1, P], F32)
    nc.gpsimd.memset(ones_row[:], 1.0)
    ones128 = const.tile([P, P], F32)
    nc.gpsimd.memset(ones128[:], 1.0)
    strictU = const.tile([P, P], F32)
    make_upper_triangular(nc, strictU[:], val=1.0, diag=False)
    ident16 = const.tile([P, P], DT)
    make_identity(nc, ident16[:])
    ident32 = const.tile([P, P], F32)
    make_identity(nc, ident32[:])

    WrT_sb = const.tile([P, KD, E], F32)
    nc.sync.dma_start(out=WrT_sb[:], in_=WrT.rearrange("(k p) e -> p k e", p=P))
    br_sb = const.tile([1, E], F32)
    nc.sync.dma_start(out=br_sb[:], in_=br[None, :])
    be_sb = const.tile([E, DOUT], F32)
    nc.sync.dma_start(out=be_sb[:], in_=be[:, :])

    xT_sb = sbig.tile([P, KD, TLOC], F32)
    nc.sync.dma_start(out=xT_sb[:], in_=xT.rearrange("(k p) t -> p k t", p=P))

    xgT = sbig.tile([P, KD, CEFF], DT)

    probs_sb = sbig.tile([P, NT * 2], F32)
    sflat = sbig.tile([P, NT * 2], I32)     # slot ids per (token tile, choice)
    Mm_all = sbig.tile([P, NT, E], F32)     # per-tile expert masks (0/1)
    gates_all = sbig.tile([P, NT, E], F32)  # dense gates (probs scattered)

    # ---- zero the slot table ----
    zero_sb = const.tile([P, NS * 2], I32)
    nc.vector.memset(zero_sb[:], 0)
    nc.sync.dma_start(out=table.rearrange("(k p) c -> p k c", p=P),
                      in_=zero_sb[:].rearrange("p (k c) -> p k c", k=NS))

    # ---- phase A: router + routing math + scatters, per token tile ----
    for i in range(NT):
        scores_ps = ps_small.tile([P, E], F32, tag="small")
        for k in range(KD):
            nc.tensor.matmul(scores_ps[:], lhsT=xT_sb[:, k, bass.ts(i, P)],
                             rhs=WrT_sb[:, k, :], start=(k == 0), stop=False)
        # + br broadcast along tokens: ones[1,P].T @ br[1,E]
        nc.tensor.matmul(scores_ps[:], lhsT=ones_row[:, :], rhs=br_sb[:, :],
                         start=False, stop=True)

        scores = work.tile([P, E], F32)
        nc.vector.tensor_copy(scores[:], scores_ps[:])

        if SUB < 2:
            nc.vector.tensor_copy(probs_sb[:, 2 * i:2 * i + 2], scores[:, 0:2])
            continue

        m8 = work.tile([P, 8], F32)
        nc.vector.max(out=m8[:], in_=scores[:])
        i8 = work.tile([P, 8], U32)
        nc.vector.max_index(out=i8[:], in_max=m8[:], in_values=scores[:])

        # probs: p1 = sigmoid(m1 - m2), p2 = 1 - p1
        d12 = work.tile([P, 1], F32)
        nc.vector.tensor_sub(d12[:], m8[:, 0:1], m8[:, 1:2])
        p1 = work.tile([P, 1], F32)
        nc.scalar.activation(p1[:], d12[:], AF.Sigmoid)
        p2 = work.tile([P, 1], F32)
        nc.vector.tensor_scalar(p2[:], p1[:], -1.0, 1.0, op0=ALU.mult, op1=ALU.add)
        nc.vector.tensor_copy(probs_sb[:, 2 * i:2 * i + 1], p1[:])
        nc.vector.tensor_copy(probs_sb[:, 2 * i + 1:2 * i + 2], p2[:])

        if SUB < 3:
            continue

        eq1 = work.tile([P, E], F32)
        nc.vector.tensor_tensor(out=eq1[:], in0=scores[:],
                                in1=m8[:, 0:1].to_broadcast([P, E]),
                                op=ALU.is_equal)
        eq2 = work.tile([P, E], F32)
        nc.vector.tensor_tensor(out=eq2[:], in0=scores[:],
                                in1=m8[:, 1:2].to_broadcast([P, E]),
                                op=ALU.is_equal)
        nc.vector.tensor_add(Mm_all[:, i, :], eq1[:], eq2[:])

        # dense gates for the bias matmul later
        g1 = work.tile([P, E], F32)
        nc.vector.tensor_scalar(g1[:], eq1[:], p1[:, 0:1], None, op0=ALU.mult)
        g2 = work.tile([P, E], F32)
        nc.vector.tensor_scalar(g2[:], eq2[:], p2[:, 0:1], None, op0=ALU.mult)
        nc.vector.tensor_add(gates_all[:, i, :], g1[:], g2[:])

        if SUB < 3.2:
            continue

        # rank within expert: prior-tile totals + strict prefix within tile
        rank_ps = ps_small.tile([P, E], F32, tag="small")
        for j in range(i):
            nc.tensor.matmul(rank_ps[:], lhsT=ones128[:], rhs=Mm_all[:, j, :],
                             start=(j == 0), stop=False)
        nc.tensor.matmul(rank_ps[:], lhsT=strictU[:], rhs=Mm_all[:, i, :],
                         start=(i == 0), stop=True)
        rank = work.tile([P, E], F32)
        nc.vector.tensor_copy(rank[:], rank_ps[:])

        if SUB < 3.4:
            continue

        rscr = work.tile([P, E], F32)
        r1 = work.tile([P, 1], F32)
        nc.vector.tensor_mul(rscr[:], rank[:], eq1[:])
        nc.vector.reduce_sum(out=r1[:], in_=rscr[:], axis=mybir.AxisListType.X)
        rscr2 = work.tile([P, E], F32)
        r2 = work.tile([P, 1], F32)
        nc.vector.tensor_mul(rscr2[:], rank[:], eq2[:])
        nc.vector.reduce_sum(out=r2[:], in_=rscr2[:], axis=mybir.AxisListType.X)

        # slot = expert * CAP + rank
        for c, (rr, pp) in enumerate(((r1, p1), (r2, p2))):
            idxf = work.tile([P, 1], F32)
            nc.vector.tensor_copy(idxf[:], i8[:, c:c + 1])
            sf = work.tile([P, 1], F32)
            nc.vector.tensor_scalar(sf[:], idxf[:], float(CAP), None, op0=ALU.mult)
            nc.vector.tensor_add(sf[:], sf[:], rr[:])
            nc.vector.tensor_copy(sflat[:, 2 * i + c:2 * i + c + 1], sf[:])

            if SUB < 4:
                continue
            packed = work.tile([P, 2], I32)
            nc.gpsimd.iota(packed[:, 0:1], pattern=[[0, 1]], base=i * P,
                           channel_multiplier=1)
            nc.vector.tensor_copy(packed[:, 1:2].bitcast(F32), pp[:])
            nc.gpsimd.indirect_dma_start(
                out=table[:, :], in_=packed[:, :],
                out_offset=bass.IndirectOffsetOnAxis(
                    ap=sflat[:, 2 * i + c:2 * i + c + 1], axis=0),
                in_offset=None,
                bounds_check=CEFF - 1, oob_is_err=False)

    if PHASES < 2:
        # dump something into out to keep outputs defined
        for i in range(NT):
            z = outp.tile([P, DOUT], F32, tag="outsb")
            nc.vector.memset(z[:], 0.0)
            nc.sync.dma_start(out=out_ap[bass.ts(i, P), :], in_=z[:])
        nc.sync.dma_start(out=probs_ap.rearrange("(i p) c -> p i c", p=P),
                          in_=probs_sb[:].rearrange("p (i c) -> p i c", i=NT))
        return

    # ---- phase B: read back slot table, gather + transpose x ----
    ids_sb = sbig.tile([P, NS * 2], I32)
    nc.sync.dma_start(out=ids_sb[:].rearrange("p (k c) -> p k c", k=NS),
                      in_=table.rearrange("(k p) c -> p k c", p=P))

    for k in range(NS):
        xg = xgp.tile([P, DIN], DT, tag="xg")
        nc.gpsimd.indirect_dma_start(
            out=xg[:], out_offset=None, in_=xh[:, :],
            in_offset=bass.IndirectOffsetOnAxis(ap=ids_sb[:, 2 * k:2 * k + 1],
                                                axis=0))
        for c in range(KD):
            txp = ps_txp.tile([P, P], DT, tag="txp")
            nc.tensor.transpose(txp[:], xg[:, bass.ts(c, P)], ident16[:])
            nc.vector.tensor_copy(xgT[:, c, bass.ts(k, P)], txp[:])

    if PHASES < 3:
        for i in range(NT):
            z = outp.tile([P, DOUT], F32, tag="outsb")
            nc.vector.memset(z[:], 0.0)
            nc.vector.tensor_copy(z[:, 0:NS * 2].bitcast(I32), ids_sb[:])
            nc.sync.dma_start(out=out_ap[bass.ts(i, P), :], in_=z[:])
        nc.sync.dma_start(out=probs_ap.rearrange("(i p) c -> p i c", p=P),
                          in_=probs_sb[:].rearrange("p (i c) -> p i c", i=NT))
        return

    # ---- phase C: expert matmuls ----
    for e in range(E):
        wet = wetp.tile([P, KD, DOUT], DT, tag="wet")
        nc.sync.dma_start(out=wet[:],
                          in_=WeT[e].rearrange("(k p) f -> p k f", p=P))
        for m in range(CAP // P):
            st = e * (CAP // P) + m           # global slot tile index
            pscale = ids_sb[:, 2 * st + 1:2 * st + 2].bitcast(F32)
            y_sb = yp.tile([P, DOUT], DT, tag="ysb")
            for n in range(ND):
                y_ps = ps_y.tile([P, NFREE], F32, tag="yps")
                for k in range(KD):
                    nc.tensor.matmul(y_ps[:],
                                     lhsT=xgT[:, k, bass.ts(st, P)],
                                     rhs=wet[:, k, bass.ts(n, NFREE)],
                                     start=(k == 0), stop=(k == KD - 1))
                nc.scalar.activation(y_sb[:, bass.ts(n, NFREE)], y_ps[:],
                                     AF.Copy, scale=pscale)
            nc.sync.dma_start(out=y_dram[bass.ts(st, P), :], in_=y_sb[:])

    if PHASES < 4:
        for i in range(NT):
            z = outp.tile([P, DOUT], F32, tag="outsb")
            nc.vector.memset(z[:], 0.0)
            nc.sync.dma_start(out=out_ap[bass.ts(i, P), :], in_=z[:])
        nc.sync.dma_start(out=probs_ap.rearrange("(i p) c -> p i c", p=P),
                          in_=probs_sb[:].rearrange("p (i c) -> p i c", i=NT))
        return

    # ---- phase D: combine per token tile ----
    for i in range(NT):
        y1 = ygp.tile([P, DOUT], DT, tag="y1")
        nc.gpsimd.indirect_dma_start(
            out=y1[:], out_offset=None, in_=y_dram[:, :],
            in_offset=bass.IndirectOffsetOnAxis(ap=sflat[:, 2 * i:2 * i + 1],
                                                axis=0))
        y2 = ygp.tile([P, DOUT], DT, tag="y2")
        nc.gpsimd.indirect_dma_start(
            out=y2[:], out_offset=None, in_=y_dram[:, :],
            in_offset=bass.IndirectOffsetOnAxis(ap=sflat[:, 2 * i + 1:2 * i + 2],
                                                axis=0))

        # bias = gates @ be, via gatesT
        gt_ps = ps_bias.tile([E, P], F32, tag="bias")
        nc.tensor.transpose(gt_ps[:], gates_all[:, i, :], ident32[:])
        gt_sb = work.tile([E, P], F32)
        nc.vector.tensor_copy(gt_sb[:], gt_ps[:])

        out_sb = outp.tile([P, DOUT], F32, tag="outsb")
        for n in range(ND):
            b_ps = ps_bias.tile([P, NFREE], F32, tag="bias")
            nc.tensor.matmul(b_ps[:], lhsT=gt_sb[:], rhs=be_sb[:, bass.ts(n, NFREE)],
                             start=True, stop=True)
            nc.vector.tensor_add(out_sb[:, bass.ts(n, NFREE)],
                                 y1[:, bass.ts(n, NFREE)], y2[:, bass.ts(n, NFREE)])
            nc.vector.tensor_add(out_sb[:, bass.ts(n, NFREE)],
                                 out_sb[:, bass.ts(n, NFREE)], b_ps[:])
        nc.sync.dma_start(out=out_ap[bass.ts(i, P), :], in_=out_sb[:])

    nc.sync.dma_start(out=probs_ap.rearrange("(i p) c -> p i c", p=P),
                      in_=probs_sb[:].rearrange("p (i c) -> p i c", i=NT))


_BUILD_CACHE = {}


def build_nc():
    if "nc" in _BUILD_CACHE:
        return _BUILD_CACHE["nc"]
    nc = bacc.Bacc("TRN2", target_bir_lowering=False, debug=False,
                   enable_asserts=False, num_devices=NCORES)
    xT = nc.dram_tensor("xT", [DIN, TLOC], F32, kind="ExternalInput").ap()
    xh = nc.dram_tensor("xh", [TLOC, DIN], DT, kind="ExternalInput").ap()
    WrT = nc.dram_tensor("WrT", [DIN, E], F32, kind="ExternalInput").ap()
    br = nc.dram_tensor("br", [E], F32, kind="ExternalInput").ap()
    WeT = nc.dram_tensor("WeT", [E, DIN, DOUT], DT, kind="ExternalInput").ap()
    be = nc.dram_tensor("be", [E, DOUT], F32, kind="ExternalInput").ap()
    out = nc.dram_tensor("out", [TLOC, DOUT], F32, kind="ExternalOutput").ap()
    probs = nc.dram_tensor("probs", [TLOC, 2], F32, kind="ExternalOutput").ap()

    with tile.TileContext(nc) as tc:
        with ExitStack() as ctx:
            emit_moe(ctx, tc, out, probs, xT, xh, WrT, br, WeT, be)
    nc.compile()
    _BUILD_CACHE["nc"] = nc
    return nc


def make_in_maps(x, Wr, br, We, be):
    x = np.asarray(x, dtype=np.float32)
    Wr = np.asarray(Wr, dtype=np.float32)
    br = np.asarray(br, dtype=np.float32)
    We = np.asarray(We, dtype=np.float32)
    be = np.asarray(be, dtype=np.float32)

    WrT = np.ascontiguousarray(Wr.T)
    WeT = np.ascontiguousarray(We.transpose(0, 2, 1)).astype(NP_DT)
    in_maps = []
    for c in range(NCORES):
        xs = x[c * TLOC:(c + 1) * TLOC]
        in_maps.append({
            "xT": np.ascontiguousarray(xs.T),
            "xh": xs.astype(NP_DT),
            "WrT": WrT,
            "br": br,
            "WeT": WeT,
            "be": be,
        })
    return in_maps


def run(x, Wr, br, We, be, trace=False):
    from concourse.bass_utils import run_bass_kernel_spmd
    nc = build_nc()
    in_maps = make_in_maps(x, Wr, br, We, be)
    res = run_bass_kernel_spmd(nc, in_maps, core_ids=list(range(NCORES)),
                               trace=trace)
    out = np.concatenate([r["out"] for r in res.results], axis=0)
    probs = np.concatenate([r["probs"] for r in res.results], axis=0)
    return (out, probs), res


def kernel(x, Wr, br, We, be):
    (out, probs), _ = run(x, Wr, br, We, be, trace=False)
    return out, probs


if __name__ == "__main__":
    nc = build_nc()
    print("built ok:", len(nc.m.functions[0].instructions) if hasattr(nc.m.functions[0], "instructions") else "n/a")
